# revision 22
# baseline (speedup 1.0000x reference)
"""2-layer GCN encoder (PyG GCNConv semantics) on 8 Trainium2 NeuronCores.

  out_l = relu(dinv * (A_hat @ u_l) + b_l),  u_l = (dinv * in_l) @ W_l
  A_hat includes self loops; dinv = deg^-1/2 (deg incl. self loop).

v2 design (group-outer L2, packed gathers, host-built L1 one-hots):

Nodes are relabelled by a degree-balancing permutation, padded to NP=100352,
and partitioned into 784 dst tiles of 128 (98 per core, 49 minis of 2 tiles).

Layer 1: host pregathers xs=x*dinv rows into per-mini packed slot streams
(deduped by (dst,src) with multiplicity folded into host-built fp8 one-hot
columns, streamed alongside). Device: per mini, scatter-add S^T per tile via
chunk matmuls (lhsT=msg chunk, rhs=one-hot col), then W1, relu/dinv epilogue,
W2 -> u2 tile; u2 slabs (14 tiles) DMA to u2_in[q]; AllGather (issued from the
vector queue) u2_in[q] -> u2_out[q].

Layer 2: loops GROUP-OUTER so dma_gather descriptor generation (the Q7
bottleneck) starts right after AllGather 0 lands and never waits for later
groups. One gather call per (group g, 14-tile span gc): edges packed
contiguously sorted by tile (no per-tile cap padding; boundary chunks get one
one-hot column per touched tile). One-hot built on DVE via is_equal from a
preloaded dloc table; scatter chunk matmuls accumulate per 4-tile psum region,
then add into a persistent SBUF fp32 accumulator. Self-loop identity matmuls
and the relu(dinv*acc) epilogue ride the last group's pass.

All per-call shapes are shared across cores (padded to the max) so one SPMD
program serves all 8 cores.
"""

import time
from contextlib import ExitStack
from dataclasses import dataclass

import numpy as np
import ml_dtypes

import concourse.bass as bass
import concourse.bacc as bacc
import concourse.mybir as mybir
import concourse.tile as tile
from concourse.bass_utils import run_bass_kernel_spmd

BF16 = ml_dtypes.bfloat16
FP8 = ml_dtypes.float8_e4m3
P = 128


@dataclass(frozen=True)
class Cfg:
    n_cores: int = 8
    d: int = 128
    n_real: int = 100000
    shard: int = 12544       # nodes per core (98 tiles)
    grp: int = 7             # source blocks (int16 rel-idx range)
    gct: int = 14            # dst tiles per gather call
    b1: int = 2              # dst tiles per L1 mini

    @property
    def np_(self):
        return self.n_cores * self.shard

    @property
    def tiles(self):
        return self.shard // P          # 98

    @property
    def minis(self):
        return self.tiles // self.b1    # 49

    @property
    def gcs(self):
        return self.tiles // self.gct   # 7 gather-call spans

    @property
    def qrows(self):
        return self.shard // self.grp   # 1792 rows per u2 block

    @property
    def cap(self):  # test.py compat (prints cfg.cap)
        return 0


FULL_CFG = Cfg()

LAST_INFO: dict = {}


def install_ntff_hook():
    """Provide antenv.axon_hooks (absent on this image) so that
    run_bass_kernel_spmd(trace=True) can capture NTFF profiles."""
    import sys
    import types

    if "antenv.axon_hooks" in sys.modules:
        return
    mod = types.ModuleType("antenv.axon_hooks")
    holder = [None]
    mod.set_axon_ntff_profile_hook = lambda h: holder.__setitem__(0, h)
    mod.get_axon_ntff_profile_hook = lambda: holder[0]
    sys.modules["antenv.axon_hooks"] = mod
    try:
        import antenv

        antenv.axon_hooks = mod
    except ImportError:
        pass
    try:
        from trn_agent_boot.trn_boot import _ntff_profile_via_ctypes

        hook = _ntff_profile_via_ctypes("/opt/axon/libaxon_pjrt.so")
        if hook is not None:
            mod.set_axon_ntff_profile_hook(hook)
    except Exception as e:  # profiling optional
        print(f"NTFF hook install failed: {e}")


def _relabel(x, edge_index, cfg: Cfg):
    """Degree-balancing node permutation (same as baseline)."""
    N = cfg.n_real
    NP = cfg.np_
    e_src = np.asarray(edge_index[0]).astype(np.int64)
    e_dst = np.asarray(edge_index[1]).astype(np.int64)
    loops = np.arange(N, dtype=np.int64)
    dst0 = np.concatenate([e_dst, loops])
    deg0 = np.bincount(dst0, minlength=N).astype(np.float32)

    ntiles = NP // P
    order_by_deg = np.argsort(-deg0, kind="stable")
    dealt = np.full(P * ntiles, -1, np.int64)
    dealt[:N] = order_by_deg
    dealt = dealt.reshape(P, ntiles)
    dealt[1::2] = dealt[1::2, ::-1]
    new_of = np.full(N, -1, np.int64)
    rr, tt = np.nonzero(dealt >= 0)
    new_ids = tt * P + rr
    new_of[dealt[rr, tt]] = new_ids
    orig_of = np.full(NP, -1, np.int64)
    orig_of[new_ids] = dealt[rr, tt]

    deg = np.zeros(NP, np.float32)
    deg[new_ids] = deg0[dealt[rr, tt]]
    dinv = np.zeros(NP, np.float32)
    nz = deg > 0
    dinv[nz] = 1.0 / np.sqrt(deg[nz])

    xs = np.zeros((NP, cfg.d), np.float32)
    dinv0 = np.zeros(N, np.float32)
    dinv0[deg0 > 0] = 1.0 / np.sqrt(deg0[deg0 > 0])
    xs[new_of] = np.asarray(x, np.float32) * dinv0[:, None]
    return (
        new_of[e_src],
        new_of[e_dst],
        new_of,
        orig_of,
        dinv,
        xs.astype(BF16),
    )


def _union_ranges(starts, ends, nch):
    """Per-tile union chunk ranges over cores.

    starts/ends: [n_cores, T] slot prefix bounds per tile; returns per-tile
    (lo, hi) chunk index ranges (union over cores), clipped to [0, nch)."""
    lo = np.min(starts // P, axis=0)
    hi = np.max((ends + P - 1) // P, axis=0)
    return np.minimum(lo, nch), np.minimum(hi, nch)


def preprocess(x, edge_index, W1, b1, W2, b2, cfg: Cfg):
    nc_, D, NP = cfg.n_cores, cfg.d, cfg.np_
    s_new, d_new, new_of, orig_of, dinv, xs_bf = _relabel(x, edge_index, cfg)

    # ---------------- layer 1 packing (edges + self loops, deduped) --------
    loops = np.arange(cfg.n_real, dtype=np.int64)
    la = new_of[loops]
    src1 = np.concatenate([s_new, la])
    dst1 = np.concatenate([d_new, la])
    key = dst1 * NP + src1
    uk, mult = np.unique(key, return_counts=True)
    d1 = uk // NP
    s1 = uk % NP
    T1 = d1 >> 7
    c1 = T1 // cfg.tiles
    tloc1 = T1 % cfg.tiles
    m1i = tloc1 // cfg.b1
    tb1 = tloc1 % cfg.b1
    # already sorted by dst (=> by (c, m, tb))
    cm = c1 * cfg.minis + m1i
    cnt_cmtb = np.bincount(cm * cfg.b1 + tb1, minlength=nc_ * cfg.minis * cfg.b1)
    cnt_cmtb = cnt_cmtb.reshape(nc_, cfg.minis, cfg.b1)
    cnt_cm = cnt_cmtb.sum(-1)
    nch1 = (cnt_cm.max(0) + P - 1) // P          # [minis] shared
    off1 = np.zeros(cfg.minis + 1, np.int64)
    off1[1:] = np.cumsum(nch1)

    # slot position within (c, m)
    starts_cm = np.zeros(nc_ * cfg.minis + 1, np.int64)
    starts_cm[1:] = np.cumsum(cnt_cm.reshape(-1))
    pos1 = np.arange(len(s1)) - starts_cm[cm]

    # union incidences per mini: tile0 chunks [0, end0), tile1 [start1, nch)
    end0 = (cnt_cmtb[:, :, 0].max(0) + P - 1) // P
    start1 = cnt_cmtb[:, :, 0].min(0) // P
    ninc1 = np.minimum(end0, nch1) + (nch1 - np.minimum(start1, nch1))
    ioff1 = np.zeros(cfg.minis + 1, np.int64)
    ioff1[1:] = np.cumsum(ninc1)
    max_nch1 = int(nch1.max())
    incmap1 = np.full((cfg.minis, max_nch1, cfg.b1), -1, np.int64)
    inc1 = []  # per mini: list of (chunk, tb)
    for m in range(cfg.minis):
        lst = [(k, 0) for k in range(min(int(end0[m]), int(nch1[m])))]
        lst += [(k, 1) for k in range(min(int(start1[m]), int(nch1[m])), int(nch1[m]))]
        assert len(lst) == ninc1[m]
        for i, (k, tb) in enumerate(lst):
            incmap1[m, k, tb] = i
        inc1.append(lst)

    cols1 = int(off1[-1])
    m1_host = np.zeros((nc_, P, cols1, D), BF16)
    m1_host[c1, pos1 % P, off1[m1i] + pos1 // P] = xs_bf[s1]
    einc1 = incmap1[m1i, pos1 // P, tb1]
    assert (einc1 >= 0).all()
    icols1 = int(ioff1[-1])
    pb1u = np.zeros((nc_, P, icols1, P), np.uint8)
    pb1u[c1, pos1 % P, ioff1[m1i] + einc1, d1 & 127] = np.minimum(mult, 255)
    pb1_host = pb1u.astype(FP8)

    # ---------------- layer 2 packing (edges only, no dedup) ---------------
    s2 = s_new
    d2 = d_new
    T2 = d2 >> 7
    c2 = T2 // cfg.tiles
    tloc2 = T2 % cfg.tiles
    gc2 = tloc2 // cfg.gct
    ti2 = tloc2 % cfg.gct
    core_s = s2 // cfg.shard
    loc_s = s2 % cfg.shard
    g2 = loc_s // cfg.qrows
    rel2 = (core_s * cfg.qrows + loc_s % cfg.qrows).astype(np.int16)

    order = np.lexsort((tloc2, gc2, g2, c2))
    c2s, g2s, gc2s, ti2s = c2[order], g2[order], gc2[order], ti2[order]
    rel2s = rel2[order]
    dl2s = (d2[order] & 127).astype(np.int64)

    ncalls = cfg.grp * cfg.gcs
    call = g2s * cfg.gcs + gc2s
    ccall = c2s * ncalls + call
    cnt_ccti = np.bincount(
        ccall * cfg.gct + ti2s, minlength=nc_ * ncalls * cfg.gct
    ).reshape(nc_, ncalls, cfg.gct)
    cnt_cc = cnt_ccti.sum(-1)
    nch2_real = (cnt_cc.max(0) + P - 1) // P     # [ncalls] shared
    # uniform call size: every half-call gathers exactly H*128 rows so one
    # num_idxs register serves all calls (per-call MOVEs caused WAR stalls)
    H = (int(nch2_real.max()) + 1) // 2
    NCH2 = 2 * H
    nch2 = np.full(ncalls, NCH2, np.int64)
    offx = np.zeros(ncalls + 1, np.int64)
    offx[1:] = np.cumsum(nch2 * (P // 16))       # idx cols

    starts_cc = np.zeros(nc_ * ncalls + 1, np.int64)
    starts_cc[1:] = np.cumsum(cnt_cc.reshape(-1))
    pos2 = np.arange(len(rel2s)) - starts_cc[ccall]

    # union incidences per call from per-tile prefix bounds
    pref = np.zeros((nc_, ncalls, cfg.gct + 1), np.int64)
    pref[:, :, 1:] = np.cumsum(cnt_ccti, axis=-1)
    inc2 = []
    incmap2 = np.full((ncalls, int(nch2.max()), cfg.gct), -1, np.int64)
    ninc2 = np.zeros(ncalls, np.int64)
    for cl in range(ncalls):
        lst = []
        for ti in range(cfg.gct):
            lo = int(pref[:, cl, ti].min() // P)
            hi = int((pref[:, cl, ti + 1].max() + P - 1) // P)
            hi = min(hi, int(nch2[cl]))
            for k in range(lo, hi):
                incmap2[cl, k, ti] = len(lst)
                lst.append((k, ti))
        inc2.append(lst)
        ninc2[cl] = len(lst)
    ioff2 = np.zeros(ncalls + 1, np.int64)
    ioff2[1:] = np.cumsum(ninc2)

    # idx table: per call [16, n/16] wrapped, replicated to 128 partitions
    idxcols = int(offx[-1])
    idx_host = np.zeros((nc_, P, idxcols), np.int16)
    for c in range(nc_):
        for cl in range(ncalls):
            n = int(nch2[cl]) * P
            a0 = starts_cc[c * ncalls + cl]
            cnt = int(cnt_cc[c, cl])
            arr = np.zeros(n, np.int16)
            arr[:cnt] = rel2s[a0 : a0 + cnt]
            wr = arr.reshape(n // 16, 16).T      # [16, n/16]
            idx_host[c, :, offx[cl] : offx[cl + 1]] = np.tile(wr, (8, 1))

    icols2 = int(ioff2[-1])
    einc2 = incmap2[call, pos2 // P, ti2s]
    assert (einc2 >= 0).all()
    pb2u = np.zeros((nc_, P, icols2, P), np.uint8)
    pb2u[c2s, pos2 % P, ioff2[call] + einc2, dl2s] = 1
    pb2_host = pb2u.astype(FP8)
    del pb2u

    # ---------------- scales / consts --------------------------------------
    dinv2 = dinv * dinv
    sc1_in = np.ascontiguousarray(
        dinv2.reshape(nc_, cfg.tiles, P).transpose(0, 2, 1)
    ).astype(np.float32)
    sc2_in = np.ascontiguousarray(
        dinv.reshape(nc_, cfg.tiles, P).transpose(0, 2, 1)
    ).astype(np.float32)
    ident_in = np.eye(P, dtype=np.float32)
    identb_in = np.eye(P, dtype=np.float32).astype(BF16)
    w1_in = np.asarray(W1, np.float32).astype(BF16)
    w2_in = np.asarray(W2, np.float32).astype(BF16)

    b1 = np.asarray(b1, np.float32)
    b2 = np.asarray(b2, np.float32)
    assert not (np.any(b1 != 0) or np.any(b2 != 0)), "bias path not implemented"

    meta = dict(
        nch1=[int(v) for v in nch1],
        off1=[int(v) for v in off1],
        inc1=inc1,
        ioff1=[int(v) for v in ioff1],
        nch2=[int(v) for v in nch2],
        offx=[int(v) for v in offx],
        inc2=inc2,
        ioff2=[int(v) for v in ioff2],
        idxcols=idxcols,
        icols2=icols2,
        cols1=cols1,
        icols1=icols1,
    )

    in_maps = []
    for c in range(nc_):
        in_maps.append(
            {
                "m1": m1_host[c].reshape(P, cols1 * D),
                "pb1": pb1_host[c].reshape(P, icols1 * P),
                "pb2": pb2_host[c].reshape(P, icols2 * P),
                "idxs": idx_host[c],
                "w1": w1_in,
                "w2": w2_in,
                "ident": ident_in,
                "identb": identb_in,
                "sc1": sc1_in[c],
                "sc2": sc2_in[c],
            }
        )
    return in_maps, meta, orig_of


def build_program(cfg: Cfg, meta: dict):
    D = cfg.d
    dt = mybir.dt
    Relu = mybir.ActivationFunctionType.Relu
    nch1, off1, inc1, ioff1 = meta["nch1"], meta["off1"], meta["inc1"], meta["ioff1"]
    nch2, offx, inc2, ioff2 = meta["nch2"], meta["offx"], meta["inc2"], meta["ioff2"]

    nc = bacc.Bacc(
        "TRN2",
        target_bir_lowering=False,
        debug=False,
        num_devices=cfg.n_cores,
        num_swdge_queues=4,
    )

    m1 = nc.dram_tensor(
        "m1", [P, meta["cols1"] * D], dt.bfloat16, kind="ExternalInput"
    ).ap()
    pb1 = nc.dram_tensor(
        "pb1", [P, meta["icols1"] * P], dt.float8e4, kind="ExternalInput"
    ).ap()
    pb2 = nc.dram_tensor(
        "pb2", [P, meta["icols2"] * P], dt.float8e4, kind="ExternalInput"
    ).ap()
    idxs = nc.dram_tensor(
        "idxs", [P, meta["idxcols"]], dt.int16, kind="ExternalInput"
    ).ap()
    w1 = nc.dram_tensor("w1", [D, D], dt.bfloat16, kind="ExternalInput").ap()
    w2 = nc.dram_tensor("w2", [D, D], dt.bfloat16, kind="ExternalInput").ap()
    ident = nc.dram_tensor("ident", [P, P], dt.float32, kind="ExternalInput").ap()
    identb = nc.dram_tensor("identb", [P, P], dt.bfloat16, kind="ExternalInput").ap()
    sc1 = nc.dram_tensor("sc1", [P, cfg.tiles], dt.float32, kind="ExternalInput").ap()
    sc2 = nc.dram_tensor("sc2", [P, cfg.tiles], dt.float32, kind="ExternalInput").ap()
    out = nc.dram_tensor("out", [cfg.shard, D], dt.float32, kind="ExternalOutput").ap()

    rg = [list(range(cfg.n_cores))]

    with tile.TileContext(nc) as tc, ExitStack() as ctx:
        const = ctx.enter_context(tc.tile_pool(name="const", bufs=1))
        dram = ctx.enter_context(tc.tile_pool(name="dram", bufs=1, space="DRAM"))
        accp = ctx.enter_context(tc.tile_pool(name="accp", bufs=1))
        m1pool = ctx.enter_context(tc.tile_pool(name="m1pool", bufs=2))
        p1pool = ctx.enter_context(tc.tile_pool(name="p1pool", bufs=2))
        m2pool = ctx.enter_context(tc.tile_pool(name="m2pool", bufs=3))
        p2pool = ctx.enter_context(tc.tile_pool(name="p2pool", bufs=2))
        upool = ctx.enter_context(tc.tile_pool(name="upool", bufs=3))
        u2sp = ctx.enter_context(tc.tile_pool(name="u2sp", bufs=2))
        opool = ctx.enter_context(tc.tile_pool(name="opool", bufs=2))
        work = ctx.enter_context(tc.tile_pool(name="work", bufs=3))
        psl1 = ctx.enter_context(tc.tile_pool(name="psl1", bufs=2, space="PSUM"))
        psl2 = ctx.enter_context(tc.tile_pool(name="psl2", bufs=2, space="PSUM"))

        # --- prologue: metadata preloads (scalar ring) + consts ------------
        idx_sb = const.tile([P, meta["idxcols"]], dt.int16)
        nc.scalar.dma_start(idx_sb[:], idxs[:])
        w1_sb = const.tile([D, D], dt.bfloat16)
        nc.sync.dma_start(w1_sb[:], w1[:])
        w2_sb = const.tile([D, D], dt.bfloat16)
        nc.sync.dma_start(w2_sb[:], w2[:])
        ident_sb = const.tile([P, P], dt.float32)
        nc.sync.dma_start(ident_sb[:], ident[:])
        identb_sb = const.tile([P, P], dt.bfloat16)
        nc.sync.dma_start(identb_sb[:], identb[:])
        sc1_sb = const.tile([P, cfg.tiles], dt.float32)
        nc.sync.dma_start(sc1_sb[:], sc1[:])
        sc2_sb = const.tile([P, cfg.tiles], dt.float32)
        nc.sync.dma_start(sc2_sb[:], sc2[:])

        acc = accp.tile([P, cfg.tiles, D], dt.float32)

        # one shared num_idxs register for every (uniform-size) gather half-call
        half_nch = nch2[0] // 2
        nid_reg = nc.gpsimd.to_reg(half_nch * P)

        u2_in = [
            dram.tile([cfg.qrows, D], dt.bfloat16, name=f"u2in{q}")
            for q in range(cfg.grp)
        ]
        u2_out = [
            dram.tile(
                [cfg.qrows * cfg.n_cores, D],
                dt.bfloat16,
                name=f"u2out{q}",
                addr_space="Shared",
            )
            for q in range(cfg.grp)
        ]

        def emit_mini(m, u2blk):
            mb1 = m1pool.tile([P, nch1[m], D], dt.bfloat16, tag="mb1")
            nc.sync.dma_start(
                mb1[:],
                m1[:, off1[m] * D : off1[m + 1] * D].rearrange(
                    "p (k d) -> p k d", d=D
                ),
            )
            pbt = p1pool.tile([P, ioff1[m + 1] - ioff1[m], P], dt.float8e4, tag="pb1")
            nc.sync.dma_start(
                pbt[:],
                pb1[:, ioff1[m] * P : ioff1[m + 1] * P].rearrange(
                    "p (k d) -> p k d", d=P
                ),
            )
            incs = inc1[m]
            for tb in range(cfg.b1):
                t = cfg.b1 * m + tb
                mine = [i for i, (k, tb_) in enumerate(incs) if tb_ == tb]
                psS = psl1.tile([P, D], dt.float32, tag="l1b")
                for j, i in enumerate(mine):
                    k = incs[i][0]
                    nc.tensor.matmul(
                        psS[:],
                        lhsT=mb1[:, k, :],
                        rhs=pbt[:, i, :],
                        start=(j == 0),
                        stop=(j == len(mine) - 1),
                    )
                sT = work.tile([P, D], dt.bfloat16, tag="sT")
                nc.scalar.copy(sT[:], psS[:])
                psA = psl1.tile([P, D], dt.float32, tag="l1b")
                nc.tensor.matmul(psA[:], lhsT=sT[:], rhs=w1_sb[:], start=True, stop=True)
                t2 = work.tile([P, D], dt.float32, tag="t2")
                nc.scalar.activation(t2[:], psA[:], Relu, scale=sc1_sb[:, t : t + 1])
                psT = psl1.tile([P, D], dt.float32, tag="l1b")
                nc.tensor.transpose(psT[:], t2[:], ident_sb[:])
                tT = work.tile([P, D], dt.bfloat16, tag="tT")
                nc.scalar.copy(tT[:], psT[:])
                psU = psl1.tile([P, D], dt.float32, tag="l1b")
                nc.tensor.matmul(psU[:], lhsT=tT[:], rhs=w2_sb[:], start=True, stop=True)
                nc.scalar.copy(u2blk[:, (m % 7) * cfg.b1 + tb, :], psU[:])

        def emit_group(g):
            for gc in range(cfg.gcs):
                cl = g * cfg.gcs + gc
                mb2 = m2pool.tile([P, nch2[cl], D], dt.bfloat16, tag="mb2")
                # split into two half-calls on different SWDGE queues so the
                # two ring drains overlap (256B/row packets drain slowly per
                # ring) and early chunks unblock their consumers sooner.
                h = half_nch
                for half, (k0, k1) in enumerate(((0, h), (h, nch2[cl]))):
                    nc.gpsimd.dma_gather(
                        mb2[:, k0:k1, :],
                        u2_out[g][:],
                        idx_sb[:, offx[cl] + k0 * 8 : offx[cl] + k1 * 8],
                        (k1 - k0) * P,
                        nid_reg,
                        D,
                        single_packet=False,
                        queue_num=(2 * cl + half) % 4,
                    )
                ninc = ioff2[cl + 1] - ioff2[cl]
                pbt2 = p2pool.tile([P, ninc, P], dt.float8e4, tag="pb2")
                nc.scalar.dma_start(
                    pbt2[:],
                    pb2[:, ioff2[cl] * P : ioff2[cl + 1] * P].rearrange(
                        "p (k d) -> p k d", d=P
                    ),
                )
                if g == cfg.grp - 1:
                    u2b = upool.tile([P, cfg.gct, D], dt.bfloat16, tag="u2b")
                    nc.sync.dma_start(
                        u2b[:],
                        u2_in[gc][:].rearrange("(t p) d -> p t d", p=P),
                    )
                incs = inc2[cl]
                for ti in range(cfg.gct):
                    ps = psl2.tile([P, D], dt.float32, tag="ps4")
                    mine = [i for i, (k, ti_) in enumerate(incs) if ti_ == ti]
                    first = True
                    if g == cfg.grp - 1:
                        nc.tensor.matmul(
                            ps[:],
                            lhsT=identb_sb[:],
                            rhs=u2b[:, ti, :],
                            start=True,
                            stop=(len(mine) == 0),
                        )
                        first = False
                    for j, i in enumerate(mine):
                        k = incs[i][0]
                        nc.tensor.matmul(
                            ps[:],
                            lhsT=pbt2[:, i, :],
                            rhs=mb2[:, k, :],
                            start=(first and j == 0),
                            stop=(j == len(mine) - 1),
                        )
                    asl = acc[:, gc * cfg.gct + ti, :]
                    if g == 0:
                        nc.vector.tensor_copy(asl, ps[:])
                    else:
                        nc.vector.tensor_add(asl, asl, ps[:])
                if g == cfg.grp - 1:
                    oall = opool.tile([P, cfg.gct, D], dt.float32, tag="oall")
                    for ti in range(cfg.gct):
                        t = gc * cfg.gct + ti
                        nc.scalar.activation(
                            oall[:, ti, :],
                            acc[:, t, :],
                            Relu,
                            scale=sc2_sb[:, t : t + 1],
                        )
                    B = cfg.gct * P
                    nc.sync.dma_start(
                        out[gc * B : (gc + 1) * B, :].rearrange(
                            "(t p) d -> p t d", p=P
                        ),
                        oall[:],
                    )

        for q in range(cfg.grp):
            u2blk = u2sp.tile([P, 7 * cfg.b1, D], dt.bfloat16, tag="u2s")
            for m in range(7 * q, 7 * q + 7):
                emit_mini(m, u2blk)
            nc.sync.dma_start(
                u2_in[q][:].rearrange("(t p) d -> p t d", p=P), u2blk[:]
            )
            # AllGather on the gpsimd queue, emitted before the group that
            # consumes it; dispatch is a trigger (collective runs on CC cores).
            nc.gpsimd.collective_compute(
                "AllGather",
                mybir.AluOpType.bypass,
                replica_groups=rg,
                ins=[u2_in[q].opt()],
                outs=[u2_out[q].opt()],
            )
            if q >= 1:
                emit_group(q - 1)
        emit_group(cfg.grp - 1)

    nc.compile()
    return nc


def run(x, edge_index, W1, b1, W2, b2, cfg: Cfg, trace: bool = False):
    if trace:
        install_ntff_hook()
    t0 = time.time()
    in_maps, meta, orig_of = preprocess(x, edge_index, W1, b1, W2, b2, cfg)
    t1 = time.time()
    nc = build_program(cfg, meta)
    t2 = time.time()
    res = run_bass_kernel_spmd(
        nc, in_maps, core_ids=list(range(cfg.n_cores)), trace=trace
    )
    t3 = time.time()
    outs = [res.results[c]["out"] for c in range(cfg.n_cores)]
    full_new = np.concatenate(outs, axis=0)
    full = np.zeros((cfg.n_real, cfg.d), np.float32)
    valid = orig_of >= 0
    full[orig_of[valid]] = full_new[valid]
    LAST_INFO.clear()
    LAST_INFO.update(
        dict(
            exec_time_ns=res.exec_time_ns,
            preprocess_s=t1 - t0,
            build_compile_s=t2 - t1,
            run_s=t3 - t2,
            cfg=cfg,
            results=res,
        )
    )
    return full


def kernel(x, edge_index, W1, b1, W2, b2):
    return run(
        np.asarray(x, np.float32),
        np.asarray(edge_index),
        np.asarray(W1, np.float32),
        np.asarray(b1, np.float32),
        np.asarray(W2, np.float32),
        np.asarray(b2, np.float32),
        FULL_CFG,
    )


# revision 26
# speedup vs baseline: 1.1980x; 1.1980x over previous
"""2-layer GCN encoder (PyG GCNConv semantics) on 8 Trainium2 NeuronCores.

  out_l = relu(dinv * (A_hat @ u_l) + b_l),  u_l = (dinv * in_l) @ W_l
  A_hat includes self loops; dinv = deg^-1/2 (deg incl. self loop).

v2 design (group-outer L2, packed gathers, host-built L1 one-hots):

Nodes are relabelled by a degree-balancing permutation, padded to NP=100352,
and partitioned into 784 dst tiles of 128 (98 per core, 49 minis of 2 tiles).

Layer 1: host pregathers xs=x*dinv rows into per-mini packed slot streams
(deduped by (dst,src) with multiplicity folded into host-built fp8 one-hot
columns, streamed alongside). Device: per mini, scatter-add S^T per tile via
chunk matmuls (lhsT=msg chunk, rhs=one-hot col), then W1, relu/dinv epilogue,
W2 -> u2 tile; u2 slabs (14 tiles) DMA to u2_in[q]; AllGather (issued from the
vector queue) u2_in[q] -> u2_out[q].

Layer 2: loops GROUP-OUTER so dma_gather descriptor generation (the Q7
bottleneck) starts right after AllGather 0 lands and never waits for later
groups. One gather call per (group g, 14-tile span gc): edges packed
contiguously sorted by tile (no per-tile cap padding; boundary chunks get one
one-hot column per touched tile). One-hot built on DVE via is_equal from a
preloaded dloc table; scatter chunk matmuls accumulate per 4-tile psum region,
then add into a persistent SBUF fp32 accumulator. Self-loop identity matmuls
and the relu(dinv*acc) epilogue ride the last group's pass.

All per-call shapes are shared across cores (padded to the max) so one SPMD
program serves all 8 cores.
"""

import time
from contextlib import ExitStack
from dataclasses import dataclass

import numpy as np
import ml_dtypes

import concourse.bass as bass
import concourse.bacc as bacc
import concourse.mybir as mybir
import concourse.tile as tile
from concourse.bass_utils import run_bass_kernel_spmd

BF16 = ml_dtypes.bfloat16
FP8 = ml_dtypes.float8_e4m3
P = 128


@dataclass(frozen=True)
class Cfg:
    n_cores: int = 8
    d: int = 128
    n_real: int = 100000
    shard: int = 12544       # nodes per core (98 tiles)
    grp: int = 7             # source blocks (int16 rel-idx range)
    gct: int = 14            # dst tiles per gather call
    b1: int = 2              # dst tiles per L1 mini

    @property
    def np_(self):
        return self.n_cores * self.shard

    @property
    def tiles(self):
        return self.shard // P          # 98

    @property
    def minis(self):
        return self.tiles // self.b1    # 49

    @property
    def gcs(self):
        return self.tiles // self.gct   # 7 gather-call spans

    @property
    def qrows(self):
        return self.shard // self.grp   # 1792 rows per u2 block

    @property
    def cap(self):  # test.py compat (prints cfg.cap)
        return 0


FULL_CFG = Cfg()

LAST_INFO: dict = {}


def install_ntff_hook():
    """Provide antenv.axon_hooks (absent on this image) so that
    run_bass_kernel_spmd(trace=True) can capture NTFF profiles."""
    import sys
    import types

    if "antenv.axon_hooks" in sys.modules:
        return
    mod = types.ModuleType("antenv.axon_hooks")
    holder = [None]
    mod.set_axon_ntff_profile_hook = lambda h: holder.__setitem__(0, h)
    mod.get_axon_ntff_profile_hook = lambda: holder[0]
    sys.modules["antenv.axon_hooks"] = mod
    try:
        import antenv

        antenv.axon_hooks = mod
    except ImportError:
        pass
    try:
        from trn_agent_boot.trn_boot import _ntff_profile_via_ctypes

        hook = _ntff_profile_via_ctypes("/opt/axon/libaxon_pjrt.so")
        if hook is not None:
            mod.set_axon_ntff_profile_hook(hook)
    except Exception as e:  # profiling optional
        print(f"NTFF hook install failed: {e}")


def _relabel(x, edge_index, cfg: Cfg):
    """Degree-balancing node permutation (same as baseline)."""
    N = cfg.n_real
    NP = cfg.np_
    e_src = np.asarray(edge_index[0]).astype(np.int64)
    e_dst = np.asarray(edge_index[1]).astype(np.int64)
    loops = np.arange(N, dtype=np.int64)
    dst0 = np.concatenate([e_dst, loops])
    deg0 = np.bincount(dst0, minlength=N).astype(np.float32)

    ntiles = NP // P
    order_by_deg = np.argsort(-deg0, kind="stable")
    dealt = np.full(P * ntiles, -1, np.int64)
    dealt[:N] = order_by_deg
    dealt = dealt.reshape(P, ntiles)
    dealt[1::2] = dealt[1::2, ::-1]
    new_of = np.full(N, -1, np.int64)
    rr, tt = np.nonzero(dealt >= 0)
    new_ids = tt * P + rr
    new_of[dealt[rr, tt]] = new_ids
    orig_of = np.full(NP, -1, np.int64)
    orig_of[new_ids] = dealt[rr, tt]

    deg = np.zeros(NP, np.float32)
    deg[new_ids] = deg0[dealt[rr, tt]]
    dinv = np.zeros(NP, np.float32)
    nz = deg > 0
    dinv[nz] = 1.0 / np.sqrt(deg[nz])

    xs = np.zeros((NP, cfg.d), np.float32)
    dinv0 = np.zeros(N, np.float32)
    dinv0[deg0 > 0] = 1.0 / np.sqrt(deg0[deg0 > 0])
    xs[new_of] = np.asarray(x, np.float32) * dinv0[:, None]
    return (
        new_of[e_src],
        new_of[e_dst],
        new_of,
        orig_of,
        dinv,
        xs.astype(BF16),
    )


def _union_ranges(starts, ends, nch):
    """Per-tile union chunk ranges over cores.

    starts/ends: [n_cores, T] slot prefix bounds per tile; returns per-tile
    (lo, hi) chunk index ranges (union over cores), clipped to [0, nch)."""
    lo = np.min(starts // P, axis=0)
    hi = np.max((ends + P - 1) // P, axis=0)
    return np.minimum(lo, nch), np.minimum(hi, nch)


def preprocess(x, edge_index, W1, b1, W2, b2, cfg: Cfg):
    nc_, D, NP = cfg.n_cores, cfg.d, cfg.np_
    s_new, d_new, new_of, orig_of, dinv, xs_bf = _relabel(x, edge_index, cfg)

    # ---------------- layer 1 packing (edges + self loops, deduped) --------
    loops = np.arange(cfg.n_real, dtype=np.int64)
    la = new_of[loops]
    src1 = np.concatenate([s_new, la])
    dst1 = np.concatenate([d_new, la])
    key = dst1 * NP + src1
    uk, mult = np.unique(key, return_counts=True)
    d1 = uk // NP
    s1 = uk % NP
    T1 = d1 >> 7
    c1 = T1 // cfg.tiles
    tloc1 = T1 % cfg.tiles
    m1i = tloc1 // cfg.b1
    tb1 = tloc1 % cfg.b1
    # already sorted by dst (=> by (c, m, tb))
    cm = c1 * cfg.minis + m1i
    cnt_cmtb = np.bincount(cm * cfg.b1 + tb1, minlength=nc_ * cfg.minis * cfg.b1)
    cnt_cmtb = cnt_cmtb.reshape(nc_, cfg.minis, cfg.b1)
    cnt_cm = cnt_cmtb.sum(-1)
    nch1 = (cnt_cm.max(0) + P - 1) // P          # [minis] shared
    off1 = np.zeros(cfg.minis + 1, np.int64)
    off1[1:] = np.cumsum(nch1)

    # slot position within (c, m)
    starts_cm = np.zeros(nc_ * cfg.minis + 1, np.int64)
    starts_cm[1:] = np.cumsum(cnt_cm.reshape(-1))
    pos1 = np.arange(len(s1)) - starts_cm[cm]

    # union incidences per mini: tile0 chunks [0, end0), tile1 [start1, nch)
    end0 = (cnt_cmtb[:, :, 0].max(0) + P - 1) // P
    start1 = cnt_cmtb[:, :, 0].min(0) // P
    ninc1 = np.minimum(end0, nch1) + (nch1 - np.minimum(start1, nch1))
    ioff1 = np.zeros(cfg.minis + 1, np.int64)
    ioff1[1:] = np.cumsum(ninc1)
    max_nch1 = int(nch1.max())
    incmap1 = np.full((cfg.minis, max_nch1, cfg.b1), -1, np.int64)
    inc1 = []  # per mini: list of (chunk, tb)
    for m in range(cfg.minis):
        lst = [(k, 0) for k in range(min(int(end0[m]), int(nch1[m])))]
        lst += [(k, 1) for k in range(min(int(start1[m]), int(nch1[m])), int(nch1[m]))]
        assert len(lst) == ninc1[m]
        for i, (k, tb) in enumerate(lst):
            incmap1[m, k, tb] = i
        inc1.append(lst)

    cols1 = int(off1[-1])
    m1_host = np.zeros((nc_, P, cols1, D), BF16)
    m1_host[c1, pos1 % P, off1[m1i] + pos1 // P] = xs_bf[s1]
    einc1 = incmap1[m1i, pos1 // P, tb1]
    assert (einc1 >= 0).all()
    icols1 = int(ioff1[-1])
    pb1u = np.zeros((nc_, P, icols1, P), np.uint8)
    pb1u[c1, pos1 % P, ioff1[m1i] + einc1, d1 & 127] = np.minimum(mult, 255)
    pb1_host = pb1u.astype(FP8)

    # ---------------- layer 2 packing (edges only, no dedup) ---------------
    s2 = s_new
    d2 = d_new
    T2 = d2 >> 7
    c2 = T2 // cfg.tiles
    tloc2 = T2 % cfg.tiles
    gc2 = tloc2 // cfg.gct
    ti2 = tloc2 % cfg.gct
    core_s = s2 // cfg.shard
    loc_s = s2 % cfg.shard
    g2 = loc_s // cfg.qrows
    rel2 = (core_s * cfg.qrows + loc_s % cfg.qrows).astype(np.int16)

    order = np.lexsort((tloc2, gc2, g2, c2))
    c2s, g2s, gc2s, ti2s = c2[order], g2[order], gc2[order], ti2[order]
    rel2s = rel2[order]
    dl2s = (d2[order] & 127).astype(np.int64)

    ncalls = cfg.grp * cfg.gcs
    call = g2s * cfg.gcs + gc2s
    ccall = c2s * ncalls + call
    cnt_ccti = np.bincount(
        ccall * cfg.gct + ti2s, minlength=nc_ * ncalls * cfg.gct
    ).reshape(nc_, ncalls, cfg.gct)
    cnt_cc = cnt_ccti.sum(-1)
    nch2 = (cnt_cc.max(0) + P - 1) // P          # [ncalls] shared
    offx = np.zeros(ncalls + 1, np.int64)
    offx[1:] = np.cumsum(nch2 * (P // 16))       # idx cols

    starts_cc = np.zeros(nc_ * ncalls + 1, np.int64)
    starts_cc[1:] = np.cumsum(cnt_cc.reshape(-1))
    pos2 = np.arange(len(rel2s)) - starts_cc[ccall]

    # union incidences per call from per-tile prefix bounds
    pref = np.zeros((nc_, ncalls, cfg.gct + 1), np.int64)
    pref[:, :, 1:] = np.cumsum(cnt_ccti, axis=-1)
    inc2 = []
    incmap2 = np.full((ncalls, int(nch2.max()), cfg.gct), -1, np.int64)
    ninc2 = np.zeros(ncalls, np.int64)
    for cl in range(ncalls):
        lst = []
        for ti in range(cfg.gct):
            lo = int(pref[:, cl, ti].min() // P)
            hi = int((pref[:, cl, ti + 1].max() + P - 1) // P)
            hi = min(hi, int(nch2[cl]))
            for k in range(lo, hi):
                incmap2[cl, k, ti] = len(lst)
                lst.append((k, ti))
        inc2.append(lst)
        ninc2[cl] = len(lst)
    ioff2 = np.zeros(ncalls + 1, np.int64)
    ioff2[1:] = np.cumsum(ninc2)

    # idx table: per call [16, n/16] wrapped, replicated to 128 partitions
    idxcols = int(offx[-1])
    idx_host = np.zeros((nc_, P, idxcols), np.int16)
    for c in range(nc_):
        for cl in range(ncalls):
            n = int(nch2[cl]) * P
            a0 = starts_cc[c * ncalls + cl]
            cnt = int(cnt_cc[c, cl])
            arr = np.zeros(n, np.int16)
            arr[:cnt] = rel2s[a0 : a0 + cnt]
            wr = arr.reshape(n // 16, 16).T      # [16, n/16]
            idx_host[c, :, offx[cl] : offx[cl + 1]] = np.tile(wr, (8, 1))

    icols2 = int(ioff2[-1])
    einc2 = incmap2[call, pos2 // P, ti2s]
    assert (einc2 >= 0).all()
    pb2u = np.zeros((nc_, P, icols2, P), np.uint8)
    pb2u[c2s, pos2 % P, ioff2[call] + einc2, dl2s] = 1
    pb2_host = pb2u.astype(FP8)
    del pb2u

    # ---------------- scales / consts --------------------------------------
    dinv2 = dinv * dinv
    sc1_in = np.ascontiguousarray(
        dinv2.reshape(nc_, cfg.tiles, P).transpose(0, 2, 1)
    ).astype(np.float32)
    sc2_in = np.ascontiguousarray(
        dinv.reshape(nc_, cfg.tiles, P).transpose(0, 2, 1)
    ).astype(np.float32)
    ident_in = np.eye(P, dtype=np.float32)
    identb_in = np.eye(P, dtype=np.float32).astype(BF16)
    w1_in = np.asarray(W1, np.float32).astype(BF16)
    w2_in = np.asarray(W2, np.float32).astype(BF16)

    b1 = np.asarray(b1, np.float32)
    b2 = np.asarray(b2, np.float32)
    assert not (np.any(b1 != 0) or np.any(b2 != 0)), "bias path not implemented"

    meta = dict(
        nch1=[int(v) for v in nch1],
        off1=[int(v) for v in off1],
        inc1=inc1,
        ioff1=[int(v) for v in ioff1],
        nch2=[int(v) for v in nch2],
        offx=[int(v) for v in offx],
        inc2=inc2,
        ioff2=[int(v) for v in ioff2],
        idxcols=idxcols,
        icols2=icols2,
        cols1=cols1,
        icols1=icols1,
    )

    in_maps = []
    for c in range(nc_):
        in_maps.append(
            {
                "m1": m1_host[c].reshape(P, cols1 * D),
                "pb1": pb1_host[c].reshape(P, icols1 * P),
                "pb2": pb2_host[c].reshape(P, icols2 * P),
                "idxs": idx_host[c],
                "w1": w1_in,
                "w2": w2_in,
                "ident": ident_in,
                "identb": identb_in,
                "sc1": sc1_in[c],
                "sc2": sc2_in[c],
            }
        )
    return in_maps, meta, orig_of


def build_program(cfg: Cfg, meta: dict):
    D = cfg.d
    dt = mybir.dt
    Relu = mybir.ActivationFunctionType.Relu
    nch1, off1, inc1, ioff1 = meta["nch1"], meta["off1"], meta["inc1"], meta["ioff1"]
    nch2, offx, inc2, ioff2 = meta["nch2"], meta["offx"], meta["inc2"], meta["ioff2"]

    nc = bacc.Bacc(
        "TRN2",
        target_bir_lowering=False,
        debug=False,
        num_devices=cfg.n_cores,
        num_swdge_queues=4,
    )

    m1 = nc.dram_tensor(
        "m1", [P, meta["cols1"] * D], dt.bfloat16, kind="ExternalInput"
    ).ap()
    pb1 = nc.dram_tensor(
        "pb1", [P, meta["icols1"] * P], dt.float8e4, kind="ExternalInput"
    ).ap()
    pb2 = nc.dram_tensor(
        "pb2", [P, meta["icols2"] * P], dt.float8e4, kind="ExternalInput"
    ).ap()
    idxs = nc.dram_tensor(
        "idxs", [P, meta["idxcols"]], dt.int16, kind="ExternalInput"
    ).ap()
    w1 = nc.dram_tensor("w1", [D, D], dt.bfloat16, kind="ExternalInput").ap()
    w2 = nc.dram_tensor("w2", [D, D], dt.bfloat16, kind="ExternalInput").ap()
    ident = nc.dram_tensor("ident", [P, P], dt.float32, kind="ExternalInput").ap()
    identb = nc.dram_tensor("identb", [P, P], dt.bfloat16, kind="ExternalInput").ap()
    sc1 = nc.dram_tensor("sc1", [P, cfg.tiles], dt.float32, kind="ExternalInput").ap()
    sc2 = nc.dram_tensor("sc2", [P, cfg.tiles], dt.float32, kind="ExternalInput").ap()
    out = nc.dram_tensor("out", [cfg.shard, D], dt.float32, kind="ExternalOutput").ap()

    rg = [list(range(cfg.n_cores))]

    with tile.TileContext(nc) as tc, ExitStack() as ctx:
        const = ctx.enter_context(tc.tile_pool(name="const", bufs=1))
        dram = ctx.enter_context(tc.tile_pool(name="dram", bufs=1, space="DRAM"))
        accp = ctx.enter_context(tc.tile_pool(name="accp", bufs=1))
        m1pool = ctx.enter_context(tc.tile_pool(name="m1pool", bufs=2))
        p1pool = ctx.enter_context(tc.tile_pool(name="p1pool", bufs=2))
        m2pool = ctx.enter_context(tc.tile_pool(name="m2pool", bufs=3))
        p2pool = ctx.enter_context(tc.tile_pool(name="p2pool", bufs=2))
        upool = ctx.enter_context(tc.tile_pool(name="upool", bufs=3))
        u2sp = ctx.enter_context(tc.tile_pool(name="u2sp", bufs=2))
        opool = ctx.enter_context(tc.tile_pool(name="opool", bufs=2))
        work = ctx.enter_context(tc.tile_pool(name="work", bufs=3))
        psl1 = ctx.enter_context(tc.tile_pool(name="psl1", bufs=2, space="PSUM"))
        psl2 = ctx.enter_context(tc.tile_pool(name="psl2", bufs=2, space="PSUM"))

        # --- prologue: metadata preloads (scalar ring) + consts ------------
        idx_sb = const.tile([P, meta["idxcols"]], dt.int16)
        nc.scalar.dma_start(idx_sb[:], idxs[:])
        w1_sb = const.tile([D, D], dt.bfloat16)
        nc.sync.dma_start(w1_sb[:], w1[:])
        w2_sb = const.tile([D, D], dt.bfloat16)
        nc.sync.dma_start(w2_sb[:], w2[:])
        ident_sb = const.tile([P, P], dt.float32)
        nc.sync.dma_start(ident_sb[:], ident[:])
        identb_sb = const.tile([P, P], dt.bfloat16)
        nc.sync.dma_start(identb_sb[:], identb[:])
        sc1_sb = const.tile([P, cfg.tiles], dt.float32)
        nc.sync.dma_start(sc1_sb[:], sc1[:])
        sc2_sb = const.tile([P, cfg.tiles], dt.float32)
        nc.sync.dma_start(sc2_sb[:], sc2[:])

        acc = accp.tile([P, cfg.tiles, D], dt.float32)

        u2_in = [
            dram.tile([cfg.qrows, D], dt.bfloat16, name=f"u2in{q}")
            for q in range(cfg.grp)
        ]
        u2_out = [
            dram.tile(
                [cfg.qrows * cfg.n_cores, D],
                dt.bfloat16,
                name=f"u2out{q}",
                addr_space="Shared",
            )
            for q in range(cfg.grp)
        ]

        def emit_mini(m, u2blk):
            mb1 = m1pool.tile([P, nch1[m], D], dt.bfloat16, tag="mb1")
            nc.sync.dma_start(
                mb1[:],
                m1[:, off1[m] * D : off1[m + 1] * D].rearrange(
                    "p (k d) -> p k d", d=D
                ),
            )
            pbt = p1pool.tile([P, ioff1[m + 1] - ioff1[m], P], dt.float8e4, tag="pb1")
            nc.sync.dma_start(
                pbt[:],
                pb1[:, ioff1[m] * P : ioff1[m + 1] * P].rearrange(
                    "p (k d) -> p k d", d=P
                ),
            )
            incs = inc1[m]
            for tb in range(cfg.b1):
                t = cfg.b1 * m + tb
                mine = [i for i, (k, tb_) in enumerate(incs) if tb_ == tb]
                psS = psl1.tile([P, D], dt.float32, tag="l1b")
                for j, i in enumerate(mine):
                    k = incs[i][0]
                    nc.tensor.matmul(
                        psS[:],
                        lhsT=mb1[:, k, :],
                        rhs=pbt[:, i, :],
                        start=(j == 0),
                        stop=(j == len(mine) - 1),
                    )
                sT = work.tile([P, D], dt.bfloat16, tag="sT")
                nc.scalar.copy(sT[:], psS[:])
                psA = psl1.tile([P, D], dt.float32, tag="l1b")
                nc.tensor.matmul(psA[:], lhsT=sT[:], rhs=w1_sb[:], start=True, stop=True)
                t2 = work.tile([P, D], dt.float32, tag="t2")
                nc.scalar.activation(t2[:], psA[:], Relu, scale=sc1_sb[:, t : t + 1])
                psT = psl1.tile([P, D], dt.float32, tag="l1b")
                nc.tensor.transpose(psT[:], t2[:], ident_sb[:])
                tT = work.tile([P, D], dt.bfloat16, tag="tT")
                nc.scalar.copy(tT[:], psT[:])
                psU = psl1.tile([P, D], dt.float32, tag="l1b")
                nc.tensor.matmul(psU[:], lhsT=tT[:], rhs=w2_sb[:], start=True, stop=True)
                nc.scalar.copy(u2blk[:, (m % 7) * cfg.b1 + tb, :], psU[:])

        def emit_group(g):
            for gc in range(cfg.gcs):
                cl = g * cfg.gcs + gc
                mb2 = m2pool.tile([P, nch2[cl], D], dt.bfloat16, tag="mb2")
                # split into four quarter-calls: all 4 SWDGE queue rings drain
                # in parallel, and each quarter is small enough (idx bytes
                # < 4096) for single_packet mode's cheaper drain.
                nq = 4
                bounds = [round(j * nch2[cl] / nq) for j in range(nq + 1)]
                for j in range(nq):
                    k0, k1 = bounds[j], bounds[j + 1]
                    if k1 == k0:
                        continue
                    n_id = (k1 - k0) * P
                    nc.gpsimd.dma_gather(
                        mb2[:, k0:k1, :],
                        u2_out[g][:],
                        idx_sb[:, offx[cl] + k0 * 8 : offx[cl] + k1 * 8],
                        n_id,
                        n_id,
                        D,
                        single_packet=False,
                        queue_num=j,
                    )
                ninc = ioff2[cl + 1] - ioff2[cl]
                pbt2 = p2pool.tile([P, ninc, P], dt.float8e4, tag="pb2")
                nc.scalar.dma_start(
                    pbt2[:],
                    pb2[:, ioff2[cl] * P : ioff2[cl + 1] * P].rearrange(
                        "p (k d) -> p k d", d=P
                    ),
                )
                if g == cfg.grp - 1:
                    u2b = upool.tile([P, cfg.gct, D], dt.bfloat16, tag="u2b")
                    nc.sync.dma_start(
                        u2b[:],
                        u2_in[gc][:].rearrange("(t p) d -> p t d", p=P),
                    )
                incs = inc2[cl]
                for ti in range(cfg.gct):
                    ps = psl2.tile([P, D], dt.float32, tag="ps4")
                    mine = [i for i, (k, ti_) in enumerate(incs) if ti_ == ti]
                    first = True
                    if g == cfg.grp - 1:
                        nc.tensor.matmul(
                            ps[:],
                            lhsT=identb_sb[:],
                            rhs=u2b[:, ti, :],
                            start=True,
                            stop=(len(mine) == 0),
                        )
                        first = False
                    for j, i in enumerate(mine):
                        k = incs[i][0]
                        nc.tensor.matmul(
                            ps[:],
                            lhsT=pbt2[:, i, :],
                            rhs=mb2[:, k, :],
                            start=(first and j == 0),
                            stop=(j == len(mine) - 1),
                        )
                    asl = acc[:, gc * cfg.gct + ti, :]
                    if g == 0:
                        nc.vector.tensor_copy(asl, ps[:])
                    else:
                        nc.vector.tensor_add(asl, asl, ps[:])
                if g == cfg.grp - 1:
                    oall = opool.tile([P, cfg.gct, D], dt.float32, tag="oall")
                    for ti in range(cfg.gct):
                        t = gc * cfg.gct + ti
                        nc.scalar.activation(
                            oall[:, ti, :],
                            acc[:, t, :],
                            Relu,
                            scale=sc2_sb[:, t : t + 1],
                        )
                    B = cfg.gct * P
                    nc.sync.dma_start(
                        out[gc * B : (gc + 1) * B, :].rearrange(
                            "(t p) d -> p t d", p=P
                        ),
                        oall[:],
                    )

        for q in range(cfg.grp):
            u2blk = u2sp.tile([P, 7 * cfg.b1, D], dt.bfloat16, tag="u2s")
            for m in range(7 * q, 7 * q + 7):
                emit_mini(m, u2blk)
            nc.sync.dma_start(
                u2_in[q][:].rearrange("(t p) d -> p t d", p=P), u2blk[:]
            )
            # AllGather on the gpsimd queue, emitted before the group that
            # consumes it; dispatch is a trigger (collective runs on CC cores).
            nc.gpsimd.collective_compute(
                "AllGather",
                mybir.AluOpType.bypass,
                replica_groups=rg,
                ins=[u2_in[q].opt()],
                outs=[u2_out[q].opt()],
            )
            if q >= 1:
                emit_group(q - 1)
        emit_group(cfg.grp - 1)

    nc.compile()
    return nc


def run(x, edge_index, W1, b1, W2, b2, cfg: Cfg, trace: bool = False):
    if trace:
        install_ntff_hook()
    t0 = time.time()
    in_maps, meta, orig_of = preprocess(x, edge_index, W1, b1, W2, b2, cfg)
    t1 = time.time()
    nc = build_program(cfg, meta)
    t2 = time.time()
    res = run_bass_kernel_spmd(
        nc, in_maps, core_ids=list(range(cfg.n_cores)), trace=trace
    )
    t3 = time.time()
    outs = [res.results[c]["out"] for c in range(cfg.n_cores)]
    full_new = np.concatenate(outs, axis=0)
    full = np.zeros((cfg.n_real, cfg.d), np.float32)
    valid = orig_of >= 0
    full[orig_of[valid]] = full_new[valid]
    LAST_INFO.clear()
    LAST_INFO.update(
        dict(
            exec_time_ns=res.exec_time_ns,
            preprocess_s=t1 - t0,
            build_compile_s=t2 - t1,
            run_s=t3 - t2,
            cfg=cfg,
            results=res,
        )
    )
    return full


def kernel(x, edge_index, W1, b1, W2, b2):
    return run(
        np.asarray(x, np.float32),
        np.asarray(edge_index),
        np.asarray(W1, np.float32),
        np.asarray(b1, np.float32),
        np.asarray(W2, np.float32),
        np.asarray(b2, np.float32),
        FULL_CFG,
    )


# revision 30
# speedup vs baseline: 1.1983x; 1.0003x over previous
"""2-layer GCN encoder (PyG GCNConv semantics) on 8 Trainium2 NeuronCores.

  out_l = relu(dinv * (A_hat @ u_l) + b_l),  u_l = (dinv * in_l) @ W_l
  A_hat includes self loops; dinv = deg^-1/2 (deg incl. self loop).

v2 design (group-outer L2, packed gathers, host-built L1 one-hots):

Nodes are relabelled by a degree-balancing permutation, padded to NP=100352,
and partitioned into 784 dst tiles of 128 (98 per core, 49 minis of 2 tiles).

Layer 1: host pregathers xs=x*dinv rows into per-mini packed slot streams
(deduped by (dst,src) with multiplicity folded into host-built fp8 one-hot
columns, streamed alongside). Device: per mini, scatter-add S^T per tile via
chunk matmuls (lhsT=msg chunk, rhs=one-hot col), then W1, relu/dinv epilogue,
W2 -> u2 tile; u2 slabs (14 tiles) DMA to u2_in[q]; AllGather (issued from the
vector queue) u2_in[q] -> u2_out[q].

Layer 2: loops GROUP-OUTER so dma_gather descriptor generation (the Q7
bottleneck) starts right after AllGather 0 lands and never waits for later
groups. One gather call per (group g, 14-tile span gc): edges packed
contiguously sorted by tile (no per-tile cap padding; boundary chunks get one
one-hot column per touched tile). One-hot built on DVE via is_equal from a
preloaded dloc table; scatter chunk matmuls accumulate per 4-tile psum region,
then add into a persistent SBUF fp32 accumulator. Self-loop identity matmuls
and the relu(dinv*acc) epilogue ride the last group's pass.

All per-call shapes are shared across cores (padded to the max) so one SPMD
program serves all 8 cores.
"""

import time
from contextlib import ExitStack
from dataclasses import dataclass

import numpy as np
import ml_dtypes

import concourse.bass as bass
import concourse.bacc as bacc
import concourse.mybir as mybir
import concourse.tile as tile
from concourse.bass_utils import run_bass_kernel_spmd

BF16 = ml_dtypes.bfloat16
FP8 = ml_dtypes.float8_e4m3
P = 128


@dataclass(frozen=True)
class Cfg:
    n_cores: int = 8
    d: int = 128
    n_real: int = 100000
    shard: int = 12544       # nodes per core (98 tiles)
    grp: int = 7             # source blocks (int16 rel-idx range)
    gct: int = 14            # dst tiles per gather call
    b1: int = 2              # dst tiles per L1 mini

    @property
    def np_(self):
        return self.n_cores * self.shard

    @property
    def tiles(self):
        return self.shard // P          # 98

    @property
    def minis(self):
        return self.tiles // self.b1    # 49

    @property
    def gcs(self):
        return self.tiles // self.gct   # 7 gather-call spans

    @property
    def qrows(self):
        return self.shard // self.grp   # 1792 rows per u2 block

    @property
    def cap(self):  # test.py compat (prints cfg.cap)
        return 0


FULL_CFG = Cfg()

LAST_INFO: dict = {}


def install_ntff_hook():
    """Provide antenv.axon_hooks (absent on this image) so that
    run_bass_kernel_spmd(trace=True) can capture NTFF profiles."""
    import sys
    import types

    if "antenv.axon_hooks" in sys.modules:
        return
    mod = types.ModuleType("antenv.axon_hooks")
    holder = [None]
    mod.set_axon_ntff_profile_hook = lambda h: holder.__setitem__(0, h)
    mod.get_axon_ntff_profile_hook = lambda: holder[0]
    sys.modules["antenv.axon_hooks"] = mod
    try:
        import antenv

        antenv.axon_hooks = mod
    except ImportError:
        pass
    try:
        from trn_agent_boot.trn_boot import _ntff_profile_via_ctypes

        hook = _ntff_profile_via_ctypes("/opt/axon/libaxon_pjrt.so")
        if hook is not None:
            mod.set_axon_ntff_profile_hook(hook)
    except Exception as e:  # profiling optional
        print(f"NTFF hook install failed: {e}")


def _relabel(x, edge_index, cfg: Cfg):
    """Degree-balancing node permutation (same as baseline)."""
    N = cfg.n_real
    NP = cfg.np_
    e_src = np.asarray(edge_index[0]).astype(np.int64)
    e_dst = np.asarray(edge_index[1]).astype(np.int64)
    loops = np.arange(N, dtype=np.int64)
    dst0 = np.concatenate([e_dst, loops])
    deg0 = np.bincount(dst0, minlength=N).astype(np.float32)

    ntiles = NP // P
    order_by_deg = np.argsort(-deg0, kind="stable")
    dealt = np.full(P * ntiles, -1, np.int64)
    dealt[:N] = order_by_deg
    dealt = dealt.reshape(P, ntiles)
    dealt[1::2] = dealt[1::2, ::-1]
    new_of = np.full(N, -1, np.int64)
    rr, tt = np.nonzero(dealt >= 0)
    new_ids = tt * P + rr
    new_of[dealt[rr, tt]] = new_ids
    orig_of = np.full(NP, -1, np.int64)
    orig_of[new_ids] = dealt[rr, tt]

    deg = np.zeros(NP, np.float32)
    deg[new_ids] = deg0[dealt[rr, tt]]
    dinv = np.zeros(NP, np.float32)
    nz = deg > 0
    dinv[nz] = 1.0 / np.sqrt(deg[nz])

    xs = np.zeros((NP, cfg.d), np.float32)
    dinv0 = np.zeros(N, np.float32)
    dinv0[deg0 > 0] = 1.0 / np.sqrt(deg0[deg0 > 0])
    xs[new_of] = np.asarray(x, np.float32) * dinv0[:, None]
    return (
        new_of[e_src],
        new_of[e_dst],
        new_of,
        orig_of,
        dinv,
        xs.astype(BF16),
    )


def _union_ranges(starts, ends, nch):
    """Per-tile union chunk ranges over cores.

    starts/ends: [n_cores, T] slot prefix bounds per tile; returns per-tile
    (lo, hi) chunk index ranges (union over cores), clipped to [0, nch)."""
    lo = np.min(starts // P, axis=0)
    hi = np.max((ends + P - 1) // P, axis=0)
    return np.minimum(lo, nch), np.minimum(hi, nch)


def preprocess(x, edge_index, W1, b1, W2, b2, cfg: Cfg):
    nc_, D, NP = cfg.n_cores, cfg.d, cfg.np_
    s_new, d_new, new_of, orig_of, dinv, xs_bf = _relabel(x, edge_index, cfg)

    # ---------------- layer 1 packing (edges + self loops, deduped) --------
    loops = np.arange(cfg.n_real, dtype=np.int64)
    la = new_of[loops]
    src1 = np.concatenate([s_new, la])
    dst1 = np.concatenate([d_new, la])
    key = dst1 * NP + src1
    uk, mult = np.unique(key, return_counts=True)
    d1 = uk // NP
    s1 = uk % NP
    T1 = d1 >> 7
    c1 = T1 // cfg.tiles
    tloc1 = T1 % cfg.tiles
    m1i = tloc1 // cfg.b1
    tb1 = tloc1 % cfg.b1
    # already sorted by dst (=> by (c, m, tb))
    cm = c1 * cfg.minis + m1i
    cnt_cmtb = np.bincount(cm * cfg.b1 + tb1, minlength=nc_ * cfg.minis * cfg.b1)
    cnt_cmtb = cnt_cmtb.reshape(nc_, cfg.minis, cfg.b1)
    cnt_cm = cnt_cmtb.sum(-1)
    nch1 = (cnt_cm.max(0) + P - 1) // P          # [minis] shared
    off1 = np.zeros(cfg.minis + 1, np.int64)
    off1[1:] = np.cumsum(nch1)

    # slot position within (c, m)
    starts_cm = np.zeros(nc_ * cfg.minis + 1, np.int64)
    starts_cm[1:] = np.cumsum(cnt_cm.reshape(-1))
    pos1 = np.arange(len(s1)) - starts_cm[cm]

    # union incidences per mini: tile0 chunks [0, end0), tile1 [start1, nch)
    end0 = (cnt_cmtb[:, :, 0].max(0) + P - 1) // P
    start1 = cnt_cmtb[:, :, 0].min(0) // P
    ninc1 = np.minimum(end0, nch1) + (nch1 - np.minimum(start1, nch1))
    ioff1 = np.zeros(cfg.minis + 1, np.int64)
    ioff1[1:] = np.cumsum(ninc1)
    max_nch1 = int(nch1.max())
    incmap1 = np.full((cfg.minis, max_nch1, cfg.b1), -1, np.int64)
    inc1 = []  # per mini: list of (chunk, tb)
    for m in range(cfg.minis):
        lst = [(k, 0) for k in range(min(int(end0[m]), int(nch1[m])))]
        lst += [(k, 1) for k in range(min(int(start1[m]), int(nch1[m])), int(nch1[m]))]
        assert len(lst) == ninc1[m]
        for i, (k, tb) in enumerate(lst):
            incmap1[m, k, tb] = i
        inc1.append(lst)

    cols1 = int(off1[-1])
    m1_host = np.zeros((nc_, P, cols1, D), BF16)
    m1_host[c1, pos1 % P, off1[m1i] + pos1 // P] = xs_bf[s1]
    einc1 = incmap1[m1i, pos1 // P, tb1]
    assert (einc1 >= 0).all()
    icols1 = int(ioff1[-1])
    pb1u = np.zeros((nc_, P, icols1, P), np.uint8)
    pb1u[c1, pos1 % P, ioff1[m1i] + einc1, d1 & 127] = np.minimum(mult, 255)
    pb1_host = pb1u.astype(FP8)

    # ---------------- layer 2 packing (edges only, no dedup) ---------------
    s2 = s_new
    d2 = d_new
    T2 = d2 >> 7
    c2 = T2 // cfg.tiles
    tloc2 = T2 % cfg.tiles
    gc2 = tloc2 // cfg.gct
    ti2 = tloc2 % cfg.gct
    core_s = s2 // cfg.shard
    loc_s = s2 % cfg.shard
    g2 = loc_s // cfg.qrows
    rel2 = (core_s * cfg.qrows + loc_s % cfg.qrows).astype(np.int16)

    order = np.lexsort((tloc2, gc2, g2, c2))
    c2s, g2s, gc2s, ti2s = c2[order], g2[order], gc2[order], ti2[order]
    rel2s = rel2[order]
    dl2s = (d2[order] & 127).astype(np.int64)

    ncalls = cfg.grp * cfg.gcs
    call = g2s * cfg.gcs + gc2s
    ccall = c2s * ncalls + call
    cnt_ccti = np.bincount(
        ccall * cfg.gct + ti2s, minlength=nc_ * ncalls * cfg.gct
    ).reshape(nc_, ncalls, cfg.gct)
    cnt_cc = cnt_ccti.sum(-1)
    nch2 = (cnt_cc.max(0) + P - 1) // P          # [ncalls] shared
    offx = np.zeros(ncalls + 1, np.int64)
    offx[1:] = np.cumsum(nch2 * (P // 16))       # idx cols

    starts_cc = np.zeros(nc_ * ncalls + 1, np.int64)
    starts_cc[1:] = np.cumsum(cnt_cc.reshape(-1))
    pos2 = np.arange(len(rel2s)) - starts_cc[ccall]

    # union incidences per call from per-tile prefix bounds
    pref = np.zeros((nc_, ncalls, cfg.gct + 1), np.int64)
    pref[:, :, 1:] = np.cumsum(cnt_ccti, axis=-1)
    inc2 = []
    incmap2 = np.full((ncalls, int(nch2.max()), cfg.gct), -1, np.int64)
    ninc2 = np.zeros(ncalls, np.int64)
    for cl in range(ncalls):
        lst = []
        for ti in range(cfg.gct):
            lo = int(pref[:, cl, ti].min() // P)
            hi = int((pref[:, cl, ti + 1].max() + P - 1) // P)
            hi = min(hi, int(nch2[cl]))
            for k in range(lo, hi):
                incmap2[cl, k, ti] = len(lst)
                lst.append((k, ti))
        inc2.append(lst)
        ninc2[cl] = len(lst)
    ioff2 = np.zeros(ncalls + 1, np.int64)
    ioff2[1:] = np.cumsum(ninc2)

    # idx table: per call [16, n/16] wrapped, replicated to 128 partitions
    idxcols = int(offx[-1])
    idx_host = np.zeros((nc_, P, idxcols), np.int16)
    for c in range(nc_):
        for cl in range(ncalls):
            n = int(nch2[cl]) * P
            a0 = starts_cc[c * ncalls + cl]
            cnt = int(cnt_cc[c, cl])
            arr = np.zeros(n, np.int16)
            arr[:cnt] = rel2s[a0 : a0 + cnt]
            wr = arr.reshape(n // 16, 16).T      # [16, n/16]
            idx_host[c, :, offx[cl] : offx[cl + 1]] = np.tile(wr, (8, 1))

    icols2 = int(ioff2[-1])
    einc2 = incmap2[call, pos2 // P, ti2s]
    assert (einc2 >= 0).all()
    pb2u = np.zeros((nc_, P, icols2, P), np.uint8)
    pb2u[c2s, pos2 % P, ioff2[call] + einc2, dl2s] = 1
    pb2_host = pb2u.astype(FP8)
    del pb2u

    # ---------------- scales / consts --------------------------------------
    dinv2 = dinv * dinv
    sc1_in = np.ascontiguousarray(
        dinv2.reshape(nc_, cfg.tiles, P).transpose(0, 2, 1)
    ).astype(np.float32)
    sc2_in = np.ascontiguousarray(
        dinv.reshape(nc_, cfg.tiles, P).transpose(0, 2, 1)
    ).astype(np.float32)
    ident_in = np.eye(P, dtype=np.float32)
    identb_in = np.eye(P, dtype=np.float32).astype(BF16)
    w1_in = np.asarray(W1, np.float32).astype(BF16)
    w2_in = np.asarray(W2, np.float32).astype(BF16)

    b1 = np.asarray(b1, np.float32)
    b2 = np.asarray(b2, np.float32)
    assert not (np.any(b1 != 0) or np.any(b2 != 0)), "bias path not implemented"

    meta = dict(
        nch1=[int(v) for v in nch1],
        off1=[int(v) for v in off1],
        inc1=inc1,
        ioff1=[int(v) for v in ioff1],
        nch2=[int(v) for v in nch2],
        offx=[int(v) for v in offx],
        inc2=inc2,
        ioff2=[int(v) for v in ioff2],
        idxcols=idxcols,
        icols2=icols2,
        cols1=cols1,
        icols1=icols1,
    )

    in_maps = []
    for c in range(nc_):
        in_maps.append(
            {
                "m1": m1_host[c].reshape(P, cols1 * D),
                "pb1": pb1_host[c].reshape(P, icols1 * P),
                "pb2": pb2_host[c].reshape(P, icols2 * P),
                "idxs": idx_host[c],
                "w1": w1_in,
                "w2": w2_in,
                "ident": ident_in,
                "identb": identb_in,
                "sc1": sc1_in[c],
                "sc2": sc2_in[c],
            }
        )
    return in_maps, meta, orig_of


def build_program(cfg: Cfg, meta: dict):
    D = cfg.d
    dt = mybir.dt
    Relu = mybir.ActivationFunctionType.Relu
    nch1, off1, inc1, ioff1 = meta["nch1"], meta["off1"], meta["inc1"], meta["ioff1"]
    nch2, offx, inc2, ioff2 = meta["nch2"], meta["offx"], meta["inc2"], meta["ioff2"]

    nc = bacc.Bacc(
        "TRN2",
        target_bir_lowering=False,
        debug=False,
        num_devices=cfg.n_cores,
        num_swdge_queues=4,
    )

    m1 = nc.dram_tensor(
        "m1", [P, meta["cols1"] * D], dt.bfloat16, kind="ExternalInput"
    ).ap()
    pb1 = nc.dram_tensor(
        "pb1", [P, meta["icols1"] * P], dt.float8e4, kind="ExternalInput"
    ).ap()
    pb2 = nc.dram_tensor(
        "pb2", [P, meta["icols2"] * P], dt.float8e4, kind="ExternalInput"
    ).ap()
    idxs = nc.dram_tensor(
        "idxs", [P, meta["idxcols"]], dt.int16, kind="ExternalInput"
    ).ap()
    w1 = nc.dram_tensor("w1", [D, D], dt.bfloat16, kind="ExternalInput").ap()
    w2 = nc.dram_tensor("w2", [D, D], dt.bfloat16, kind="ExternalInput").ap()
    ident = nc.dram_tensor("ident", [P, P], dt.float32, kind="ExternalInput").ap()
    identb = nc.dram_tensor("identb", [P, P], dt.bfloat16, kind="ExternalInput").ap()
    sc1 = nc.dram_tensor("sc1", [P, cfg.tiles], dt.float32, kind="ExternalInput").ap()
    sc2 = nc.dram_tensor("sc2", [P, cfg.tiles], dt.float32, kind="ExternalInput").ap()
    out = nc.dram_tensor("out", [cfg.shard, D], dt.float32, kind="ExternalOutput").ap()

    rg = [list(range(cfg.n_cores))]

    with tile.TileContext(nc) as tc, ExitStack() as ctx:
        const = ctx.enter_context(tc.tile_pool(name="const", bufs=1))
        dram = ctx.enter_context(tc.tile_pool(name="dram", bufs=1, space="DRAM"))
        accp = ctx.enter_context(tc.tile_pool(name="accp", bufs=1))
        m1pool = ctx.enter_context(tc.tile_pool(name="m1pool", bufs=2))
        p1pool = ctx.enter_context(tc.tile_pool(name="p1pool", bufs=2))
        m2pool = ctx.enter_context(tc.tile_pool(name="m2pool", bufs=4))
        p2pool = ctx.enter_context(tc.tile_pool(name="p2pool", bufs=2))
        upool = ctx.enter_context(tc.tile_pool(name="upool", bufs=3))
        u2sp = ctx.enter_context(tc.tile_pool(name="u2sp", bufs=2))
        opool = ctx.enter_context(tc.tile_pool(name="opool", bufs=2))
        work = ctx.enter_context(tc.tile_pool(name="work", bufs=3))
        psl1 = ctx.enter_context(tc.tile_pool(name="psl1", bufs=2, space="PSUM"))
        psl2 = ctx.enter_context(tc.tile_pool(name="psl2", bufs=2, space="PSUM"))

        # --- prologue: metadata preloads (scalar ring) + consts ------------
        idx_sb = const.tile([P, meta["idxcols"]], dt.int16)
        nc.scalar.dma_start(idx_sb[:], idxs[:])
        w1_sb = const.tile([D, D], dt.bfloat16)
        nc.sync.dma_start(w1_sb[:], w1[:])
        w2_sb = const.tile([D, D], dt.bfloat16)
        nc.sync.dma_start(w2_sb[:], w2[:])
        ident_sb = const.tile([P, P], dt.float32)
        nc.sync.dma_start(ident_sb[:], ident[:])
        identb_sb = const.tile([P, P], dt.bfloat16)
        nc.sync.dma_start(identb_sb[:], identb[:])
        sc1_sb = const.tile([P, cfg.tiles], dt.float32)
        nc.sync.dma_start(sc1_sb[:], sc1[:])
        sc2_sb = const.tile([P, cfg.tiles], dt.float32)
        nc.sync.dma_start(sc2_sb[:], sc2[:])

        acc = accp.tile([P, cfg.tiles, D], dt.float32)

        u2_in = [
            dram.tile([cfg.qrows, D], dt.bfloat16, name=f"u2in{q}")
            for q in range(cfg.grp)
        ]
        u2_out = [
            dram.tile(
                [cfg.qrows * cfg.n_cores, D],
                dt.bfloat16,
                name=f"u2out{q}",
                addr_space="Shared",
            )
            for q in range(cfg.grp)
        ]

        def emit_mini(m, u2blk):
            mb1 = m1pool.tile([P, nch1[m], D], dt.bfloat16, tag="mb1")
            nc.sync.dma_start(
                mb1[:],
                m1[:, off1[m] * D : off1[m + 1] * D].rearrange(
                    "p (k d) -> p k d", d=D
                ),
            )
            pbt = p1pool.tile([P, ioff1[m + 1] - ioff1[m], P], dt.float8e4, tag="pb1")
            nc.sync.dma_start(
                pbt[:],
                pb1[:, ioff1[m] * P : ioff1[m + 1] * P].rearrange(
                    "p (k d) -> p k d", d=P
                ),
            )
            incs = inc1[m]
            for tb in range(cfg.b1):
                t = cfg.b1 * m + tb
                mine = [i for i, (k, tb_) in enumerate(incs) if tb_ == tb]
                psS = psl1.tile([P, D], dt.float32, tag="l1b")
                for j, i in enumerate(mine):
                    k = incs[i][0]
                    nc.tensor.matmul(
                        psS[:],
                        lhsT=mb1[:, k, :],
                        rhs=pbt[:, i, :],
                        start=(j == 0),
                        stop=(j == len(mine) - 1),
                    )
                sT = work.tile([P, D], dt.bfloat16, tag="sT")
                nc.scalar.copy(sT[:], psS[:])
                psA = psl1.tile([P, D], dt.float32, tag="l1b")
                nc.tensor.matmul(psA[:], lhsT=sT[:], rhs=w1_sb[:], start=True, stop=True)
                t2 = work.tile([P, D], dt.float32, tag="t2")
                nc.scalar.activation(t2[:], psA[:], Relu, scale=sc1_sb[:, t : t + 1])
                psT = psl1.tile([P, D], dt.float32, tag="l1b")
                nc.tensor.transpose(psT[:], t2[:], ident_sb[:])
                tT = work.tile([P, D], dt.bfloat16, tag="tT")
                nc.scalar.copy(tT[:], psT[:])
                psU = psl1.tile([P, D], dt.float32, tag="l1b")
                nc.tensor.matmul(psU[:], lhsT=tT[:], rhs=w2_sb[:], start=True, stop=True)
                nc.scalar.copy(u2blk[:, (m % 7) * cfg.b1 + tb, :], psU[:])

        def emit_ag(q):
            # dispatch is a trigger; the collective runs on the CC cores
            nc.gpsimd.collective_compute(
                "AllGather",
                mybir.AluOpType.bypass,
                replica_groups=rg,
                ins=[u2_in[q].opt()],
                outs=[u2_out[q].opt()],
            )

        def emit_group(g, ag_next=None):
            for gc in range(cfg.gcs):
                cl = g * cfg.gcs + gc
                mb2 = m2pool.tile([P, nch2[cl], D], dt.bfloat16, tag="mb2")
                # split into four quarter-calls: all 4 SWDGE queue rings drain
                # in parallel, and each quarter is small enough (idx bytes
                # < 4096) for single_packet mode's cheaper drain.
                nq = 4
                bounds = [round(j * nch2[cl] / nq) for j in range(nq + 1)]
                for j in range(nq):
                    k0, k1 = bounds[j], bounds[j + 1]
                    if k1 == k0:
                        continue
                    n_id = (k1 - k0) * P
                    nc.gpsimd.dma_gather(
                        mb2[:, k0:k1, :],
                        u2_out[g][:],
                        idx_sb[:, offx[cl] + k0 * 8 : offx[cl] + k1 * 8],
                        n_id,
                        n_id,
                        D,
                        single_packet=False,
                        queue_num=j,
                    )
                ninc = ioff2[cl + 1] - ioff2[cl]
                pbt2 = p2pool.tile([P, ninc, P], dt.float8e4, tag="pb2")
                nc.scalar.dma_start(
                    pbt2[:],
                    pb2[:, ioff2[cl] * P : ioff2[cl + 1] * P].rearrange(
                        "p (k d) -> p k d", d=P
                    ),
                )
                if g == cfg.grp - 1:
                    u2b = upool.tile([P, cfg.gct, D], dt.bfloat16, tag="u2b")
                    nc.sync.dma_start(
                        u2b[:],
                        u2_in[gc][:].rearrange("(t p) d -> p t d", p=P),
                    )
                incs = inc2[cl]
                for ti in range(cfg.gct):
                    ps = psl2.tile([P, D], dt.float32, tag="ps4")
                    mine = [i for i, (k, ti_) in enumerate(incs) if ti_ == ti]
                    first = True
                    if g == cfg.grp - 1:
                        nc.tensor.matmul(
                            ps[:],
                            lhsT=identb_sb[:],
                            rhs=u2b[:, ti, :],
                            start=True,
                            stop=(len(mine) == 0),
                        )
                        first = False
                    for j, i in enumerate(mine):
                        k = incs[i][0]
                        nc.tensor.matmul(
                            ps[:],
                            lhsT=pbt2[:, i, :],
                            rhs=mb2[:, k, :],
                            start=(first and j == 0),
                            stop=(j == len(mine) - 1),
                        )
                    asl = acc[:, gc * cfg.gct + ti, :]
                    if g == 0:
                        nc.vector.tensor_copy(asl, ps[:])
                    else:
                        nc.vector.tensor_add(asl, asl, ps[:])
                if gc == 0 and ag_next is not None:
                    emit_ag(ag_next)
                if g == cfg.grp - 1:
                    oall = opool.tile([P, cfg.gct, D], dt.float32, tag="oall")
                    for ti in range(cfg.gct):
                        t = gc * cfg.gct + ti
                        nc.scalar.activation(
                            oall[:, ti, :],
                            acc[:, t, :],
                            Relu,
                            scale=sc2_sb[:, t : t + 1],
                        )
                    B = cfg.gct * P
                    nc.sync.dma_start(
                        out[gc * B : (gc + 1) * B, :].rearrange(
                            "(t p) d -> p t d", p=P
                        ),
                        oall[:],
                    )

        for q in range(cfg.grp):
            u2blk = u2sp.tile([P, 7 * cfg.b1, D], dt.bfloat16, tag="u2s")
            for m in range(7 * q, 7 * q + 7):
                emit_mini(m, u2blk)
            nc.sync.dma_start(
                u2_in[q][:].rearrange("(t p) d -> p t d", p=P), u2blk[:]
            )
            if q == 0:
                emit_ag(0)
            else:
                # AG_q is emitted inside group q-1, right after its first
                # call, so group 0 starts as soon as AG0 lands while later
                # AGs still get ~a full group of lead time.
                emit_group(q - 1, ag_next=q)
        emit_group(cfg.grp - 1)

    nc.compile()
    return nc


def run(x, edge_index, W1, b1, W2, b2, cfg: Cfg, trace: bool = False):
    if trace:
        install_ntff_hook()
    t0 = time.time()
    in_maps, meta, orig_of = preprocess(x, edge_index, W1, b1, W2, b2, cfg)
    t1 = time.time()
    nc = build_program(cfg, meta)
    t2 = time.time()
    res = run_bass_kernel_spmd(
        nc, in_maps, core_ids=list(range(cfg.n_cores)), trace=trace
    )
    t3 = time.time()
    outs = [res.results[c]["out"] for c in range(cfg.n_cores)]
    full_new = np.concatenate(outs, axis=0)
    full = np.zeros((cfg.n_real, cfg.d), np.float32)
    valid = orig_of >= 0
    full[orig_of[valid]] = full_new[valid]
    LAST_INFO.clear()
    LAST_INFO.update(
        dict(
            exec_time_ns=res.exec_time_ns,
            preprocess_s=t1 - t0,
            build_compile_s=t2 - t1,
            run_s=t3 - t2,
            cfg=cfg,
            results=res,
        )
    )
    return full


def kernel(x, edge_index, W1, b1, W2, b2):
    return run(
        np.asarray(x, np.float32),
        np.asarray(edge_index),
        np.asarray(W1, np.float32),
        np.asarray(b1, np.float32),
        np.asarray(W2, np.float32),
        np.asarray(b2, np.float32),
        FULL_CFG,
    )
